# revision 3
# baseline (speedup 1.0000x reference)
"""KoLeo loss kernel for 8 trn2 NeuronCores — v2 (fp8 DoubleRow).

Math: L2-normalize rows of X [16384,768]; per row find max cosine sim
(self excluded); loss = -mean(log(sqrt(2-2*smax))).  Only the per-row max
dot is needed (indices never enter the loss).

Per core c (SPMD, core differences only via input data):
  - normalize all 16384 rows -> xn8 = fp8e4(32 * x / ||x||), staged to DRAM.
  - u16-bitcast DMA-transpose stages XT packed: partition p of 256-chunk k
    holds fp8 pair (d=256k+2p, 256k+2p+1) interleaved along free dim.
  - queries (rows [2048c, 2048(c+1)) passed as input Q) staged the same way,
    then deinterleaved on-chip to clean DoubleRow weight layout
    [128, k, e, m] (walrus rejects byte-paired weights; packed rhs is fine).
  - gram sweep: 16 m-tiles x 8 key-blocks of 2048; 3 DoubleRow fp8 matmuls
    x 4 n-blocks of 512 per block; dots scaled by 1024 in PSUM fp32.
  - self-sim removed by one extra bf16 matmul per (m,kb) adding NEGMASK
    (per-core input: -4096*I at kb==c else 0) at in-block offset 128*m.
  - row max: per (m,kb) either direct DVE reduce_max from PSUM, or ACT
    copy PSUM->SBUF fp16 then DVE tensor_tensor max into a per-m running
    acc (2x DVE mode).  Final per-m reduce + sqrt/log on device.
Host: loss = -mean(LI over all cores).
"""

import os

import ml_dtypes
import numpy as np

import concourse.bacc as bacc
import concourse.mybir as mybir
import concourse.tile as tile
from concourse.bass_utils import run_bass_kernel_spmd

F32 = mybir.dt.float32
BF16 = mybir.dt.bfloat16
FP16 = mybir.dt.float16
FP8 = mybir.dt.float8e4
U16 = mybir.dt.uint16

N = 16384
D = 768
NCORES = 8
QPC = N // NCORES          # 2048 queries per core
MT = QPC // 128            # 16 m-tiles
KB = N // 2048             # 8 key blocks of 2048
KC = 3                     # DoubleRow contraction chunks of 256
SCALE = 32.0               # fp8 quantization scale; dots scale by 1024
DSCALE = SCALE * SCALE

# per-(m,kb) fold path: direct-from-PSUM DVE reduce vs ACT copy + DVE
# tensor_tensor max into the per-m accumulator.  The direct fraction is
# skewed high for early kbs (DVE idles while ACT does staging squares) and
# low for late kbs (ACT frees up once staging ends, DVE owns the finals).
_NDIR = [8, 8, 7, 7, 7, 7, 6, 6]


def _is_direct(m, kb):
    return (m * 5 + kb) % 16 < _NDIR[kb]

LAST_EXEC_NS = None


def _build_nc():
    nc = bacc.Bacc("TRN2")

    X = nc.dram_tensor("X", [N, D], F32, kind="ExternalInput")
    Q = nc.dram_tensor("Q", [QPC, D], F32, kind="ExternalInput")
    IDENT = nc.dram_tensor("IDENT", [128, 128], BF16, kind="ExternalInput")
    NEGMASK = nc.dram_tensor("NEGMASK", [128, KB * 128], BF16, kind="ExternalInput")
    LI = nc.dram_tensor("LI", [128, MT], F32, kind="ExternalOutput")

    with tile.TileContext(nc) as tc:
        with (
            tc.tile_pool(name="dram", bufs=1, space="DRAM") as dpool,
            tc.tile_pool(name="pre", bufs=7) as pre,
            tc.tile_pool(name="sqp", bufs=1) as sqp,
            tc.tile_pool(name="stat", bufs=8) as stat,
            tc.tile_pool(name="persist", bufs=1) as persist,
            tc.tile_pool(name="psum", bufs=3, space="PSUM") as psum_pool,
            tc.tile_pool(name="small", bufs=4) as small,
        ):
            XN8 = dpool.tile([N, D], FP8)
            QN8 = dpool.tile([QPC, D], FP8)

            ident = persist.tile([128, 128], BF16, tag="ident")
            nc.sync.dma_start(ident, IDENT[:, :])
            negmask = persist.tile([128, KB * 128], BF16, tag="negmask")
            nc.sync.dma_start(negmask, NEGMASK[:, :])

            # ---- normalize + quantize rows -> fp8 (scale 32/||x||) ----
            # reads batched 2 row-tiles per DMA (fewer SP-SEQ/HWDGE slots);
            # fp8 writes issue from the idle Pool sequencer (SWDGE) so their
            # waits never block the SP queue head.
            def norm_tiles(src, dst, ntiles, quant_dve=False,
                           square_dve=False):
                # fp8 outputs of two consecutive tiles share one SWDGE write
                # (halves Pool descriptor-generation work)
                assert ntiles % 2 == 0
                for t in range(ntiles):
                    xt = pre.tile([128, D], F32, tag="xt")
                    nc.sync.dma_start(xt, src[t * 128:(t + 1) * 128, :])
                    x8 = pre.tile([128, D], FP8, tag="x8")
                    sq = sqp.tile([128, D], F32, tag="sq")
                    n2 = stat.tile([128, 1], F32, tag="n2")
                    nc.scalar.activation(
                        sq, xt, mybir.ActivationFunctionType.Square,
                        accum_out=n2)
                    sn = stat.tile([128, 1], F32, tag="sn")
                    # sn = sqrt(n2/1024) = ||x||/32
                    nc.scalar.activation(
                        sn, n2, mybir.ActivationFunctionType.Sqrt,
                        scale=1.0 / (SCALE * SCALE))
                    rs = stat.tile([128, 1], F32, tag="rs")
                    nc.vector.reciprocal(rs, sn)
                    qeng = nc.vector if quant_dve else nc.gpsimd
                    qeng.tensor_scalar(
                        x8, xt, rs, None, op0=mybir.AluOpType.mult)
                    nc.gpsimd.dma_start(dst[t * 128:(t + 1) * 128, :], x8)

            # query staging first (own slab), in a scoped pool so its SBUF is
            # released before the big accumulators allocate.
            QT8 = persist.tile([128, KC * 2 * QPC], FP8, tag="qt8")
            with tc.tile_pool(name="qstage", bufs=1) as qstage:
                norm_tiles(Q, QN8, QPC // 128, quant_dve=True, square_dve=True)
                qt_pk = qstage.tile([128, KC * QPC], U16, tag="qt_pk")
                qn8_u16 = QN8.bitcast(U16)
                for k in range(KC):
                    nc.sync.dma_start_transpose(
                        qt_pk[:, k * QPC:(k + 1) * QPC],
                        qn8_u16[:, k * 128:(k + 1) * 128],
                    )
                qtv = qt_pk.bitcast(FP8).rearrange(
                    "p (k m e) -> p k e m", k=KC, e=2)
                qt8v = QT8.rearrange("p (k e m) -> p k e m", k=KC, e=2)
                for k in range(KC):
                    for e in range(2):
                        nc.vector.tensor_copy(qt8v[:, k, e, :], qtv[:, k, e, :])

            # ---- normalize X and stage transposed keys, kb-granular ----
            XT = persist.tile([128, KC * N], U16, tag="xt_pk")
            xn8_u16 = XN8.bitcast(U16)

            def stage_kb(kb):
                norm_tiles(
                    X[kb * 2048:(kb + 1) * 2048, :],
                    XN8[kb * 2048:(kb + 1) * 2048, :],
                    2048 // 128,
                    quant_dve=(kb == 0),
                    square_dve=False,
                )
                for k in range(KC):
                    nc.sync.dma_start_transpose(
                        XT[:, k * N + kb * 2048: k * N + (kb + 1) * 2048],
                        xn8_u16[kb * 2048:(kb + 1) * 2048,
                                k * 128:(k + 1) * 128],
                    )

            # ---- main sweep ----
            ACC = persist.tile([128, MT * 1024], FP16, tag="acc")
            BM = persist.tile([128, MT * KB * 2], F32, tag="bm")
            nc.vector.memset(BM, -65000.0)
            SM = persist.tile([128, MT], F32, tag="sm")

            xtv = XT.bitcast(FP8).rearrange("p (k j e) -> p k e j", k=KC, e=2)
            qv = QT8.rearrange("p (k e m) -> p k e m", k=KC, e=2)

            first_act_done = [False] * MT
            last_act_kb = [max((kb for kb in range(KB) if not _is_direct(m, kb)),
                               default=-1) for m in range(MT)]

            with tc.tile_pool(name="cp", bufs=7) as cpool:
                stage_kb(0)
                for kb in range(KB):
                    # emit next staging ahead of this kb's sweep so its ops
                    # get earlier scheduler priority and overlap the sweep
                    # (one ahead: staging a kb is faster than sweeping one)
                    if kb + 1 < KB:
                        stage_kb(kb + 1)
                    for m in range(MT):
                        # two half-blocks of 1024 keys -> 4 psum bufs in
                        # flight (deeper MM/fold pipeline on the 8 banks)
                        for h in range(2):
                            ps = psum_pool.tile([128, 1024], F32, tag="ps")
                            for k in range(KC):
                                lhsT = qv[:, k, :, m * 128:(m + 1) * 128]
                                for nb in range(2):
                                    j0 = kb * 2048 + h * 1024 + nb * 512
                                    nc.tensor.matmul(
                                        ps[:, nb * 512:(nb + 1) * 512],
                                        lhsT,
                                        xtv[:, k, :, j0:j0 + 512],
                                        start=(k == 0),
                                        stop=(k == KC - 1),
                                        perf_mode=mybir.MatmulPerfMode.DoubleRow,
                                    )
                            # self-sim mask: adds NEGMASK[:, kb] (=-4096*I
                            # iff kb==c) at in-block cols [128*m, 128*m+128)
                            if (m * 128) // 1024 == h:
                                nc.tensor.matmul(
                                    ps[:, (m * 128) % 1024:(m * 128) % 1024 + 128],
                                    ident,
                                    negmask[:, kb * 128:(kb + 1) * 128],
                                    start=False,
                                    stop=True,
                                    skip_group_check=True,
                                )
                            if _is_direct(m, kb):
                                nc.vector.reduce_max(
                                    BM[:, (m * KB + kb) * 2 + h:
                                       (m * KB + kb) * 2 + h + 1], ps,
                                    axis=mybir.AxisListType.X)
                            elif not first_act_done[m]:
                                first_act_done[m] = True
                                nc.scalar.activation(
                                    ACC[:, m * 1024:(m + 1) * 1024], ps,
                                    mybir.ActivationFunctionType.Copy)
                            else:
                                cp = cpool.tile([128, 1024], FP16, tag="cp")
                                nc.scalar.activation(
                                    cp, ps, mybir.ActivationFunctionType.Copy)
                                nc.vector.tensor_tensor(
                                    out=ACC[:, m * 1024:(m + 1) * 1024],
                                    in0=cp,
                                    in1=ACC[:, m * 1024:(m + 1) * 1024],
                                    op=mybir.AluOpType.max)
                        if kb == last_act_kb[m]:
                            # acc complete for this m: reduce it now so the
                            # final reduces spread across the sweep
                            slot = (m * KB + kb) * 2
                            nc.vector.reduce_max(
                                BM[:, slot:slot + 1],
                                ACC[:, m * 1024:(m + 1) * 1024],
                                axis=mybir.AxisListType.X)
                        if kb == KB - 1:
                            # tiny: fold BM row (acc max lives in col 0 of
                            # this m's BM slice; direct cols fill the rest)
                            nc.vector.reduce_max(
                                SM[:, m:m + 1],
                                BM[:, m * KB * 2:(m + 1) * KB * 2],
                                axis=mybir.AxisListType.X)

            # dd = sqrt(2 - 2*smax/1024); li = log(dd + 1e-8)
            dd = persist.tile([128, MT], F32, tag="dd")
            b2 = persist.tile([128, 1], F32, tag="b2")
            nc.vector.memset(b2, 2.0)
            nc.scalar.activation(
                dd, SM, mybir.ActivationFunctionType.Sqrt,
                scale=-2.0 / DSCALE, bias=b2[:, 0:1])
            lg = persist.tile([128, MT], F32, tag="lg")
            beps = persist.tile([128, 1], F32, tag="beps")
            nc.vector.memset(beps, 1e-8)
            nc.scalar.activation(
                lg, dd, mybir.ActivationFunctionType.Ln, bias=beps[:, 0:1])
            nc.sync.dma_start(LI[:, :], lg)
    nc.compile()
    return nc


_CACHED = {}


def _get_nc():
    if "nc" not in _CACHED:
        _CACHED["nc"] = _build_nc()
    return _CACHED["nc"]


def kernel(X: np.ndarray) -> np.ndarray:
    global LAST_EXEC_NS
    X = np.ascontiguousarray(np.asarray(X, dtype=np.float32))
    assert X.shape == (N, D)

    nc = _get_nc()

    eye = np.eye(128, dtype=ml_dtypes.bfloat16)
    in_maps = []
    for c in range(NCORES):
        negmask = np.zeros((128, KB * 128), dtype=ml_dtypes.bfloat16)
        negmask[:, c * 128:(c + 1) * 128] = (
            np.eye(128) * -4096.0).astype(ml_dtypes.bfloat16)
        in_maps.append({
            "X": X,
            "Q": np.ascontiguousarray(X[c * QPC:(c + 1) * QPC]),
            "IDENT": eye,
            "NEGMASK": negmask,
        })

    res = run_bass_kernel_spmd(nc, in_maps, core_ids=list(range(NCORES)))
    LAST_EXEC_NS = res.exec_time_ns
    if LAST_EXEC_NS is None and "sim_ns" in _CACHED:
        LAST_EXEC_NS = _CACHED["sim_ns"]

    li = np.concatenate(
        [r["LI"].reshape(128, MT) for r in res.results], axis=1)
    loss = -np.float32(np.mean(li))
    return np.asarray(loss, dtype=np.float32)


def sim_exec_ns() -> float:
    """Single-core predicted duration from the TimelineSim cost model."""
    from concourse.timeline_sim import TimelineSim
    nc = _get_nc()
    sim = TimelineSim(nc, trace=False, no_exec=True)
    ns = sim.simulate()
    _CACHED["sim_ns"] = int(ns)
    return ns


if __name__ == "__main__":
    print("sim ns:", sim_exec_ns())


# revision 4
# speedup vs baseline: 1.0590x; 1.0590x over previous
"""KoLeo loss kernel for 8 trn2 NeuronCores — v2 (fp8 DoubleRow).

Math: L2-normalize rows of X [16384,768]; per row find max cosine sim
(self excluded); loss = -mean(log(sqrt(2-2*smax))).  Only the per-row max
dot is needed (indices never enter the loss).

Per core c (SPMD, core differences only via input data):
  - normalize all 16384 rows -> xn8 = fp8e4(32 * x / ||x||), staged to DRAM.
  - u16-bitcast DMA-transpose stages XT packed: partition p of 256-chunk k
    holds fp8 pair (d=256k+2p, 256k+2p+1) interleaved along free dim.
  - queries (rows [2048c, 2048(c+1)) passed as input Q) staged the same way,
    then deinterleaved on-chip to clean DoubleRow weight layout
    [128, k, e, m] (walrus rejects byte-paired weights; packed rhs is fine).
  - gram sweep: 16 m-tiles x 8 key-blocks of 2048; 3 DoubleRow fp8 matmuls
    x 4 n-blocks of 512 per block; dots scaled by 1024 in PSUM fp32.
  - self-sim removed by one extra bf16 matmul per (m,kb) adding NEGMASK
    (per-core input: -4096*I at kb==c else 0) at in-block offset 128*m.
  - row max: per (m,kb) either direct DVE reduce_max from PSUM, or ACT
    copy PSUM->SBUF fp16 then DVE tensor_tensor max into a per-m running
    acc (2x DVE mode).  Final per-m reduce + sqrt/log on device.
Host: loss = -mean(LI over all cores).
"""

import os

import ml_dtypes
import numpy as np

import concourse.bacc as bacc
import concourse.mybir as mybir
import concourse.tile as tile
from concourse.bass_utils import run_bass_kernel_spmd

F32 = mybir.dt.float32
BF16 = mybir.dt.bfloat16
FP16 = mybir.dt.float16
FP8 = mybir.dt.float8e4
U16 = mybir.dt.uint16

N = 16384
D = 768
NCORES = 8
QPC = N // NCORES          # 2048 queries per core
MT = QPC // 128            # 16 m-tiles
KB = N // 2048             # 8 key blocks of 2048
KC = 3                     # DoubleRow contraction chunks of 256
SCALE = 32.0               # fp8 quantization scale; dots scale by 1024
DSCALE = SCALE * SCALE

# per-(m,kb) fold path: direct-from-PSUM DVE reduce vs ACT copy + DVE
# tensor_tensor max into the per-m accumulator.  The direct fraction is
# skewed high for early kbs (DVE idles while ACT does staging squares) and
# low for late kbs (ACT frees up once staging ends, DVE owns the finals).
_NDIR = [8, 8, 7, 7, 7, 7, 6, 6]


def _is_direct(m, kb):
    return (m * 5 + kb) % 16 < _NDIR[kb]

LAST_EXEC_NS = None


def _build_nc():
    nc = bacc.Bacc("TRN2")

    X = nc.dram_tensor("X", [N, D], F32, kind="ExternalInput")
    Q = nc.dram_tensor("Q", [QPC, D], F32, kind="ExternalInput")
    IDENT = nc.dram_tensor("IDENT", [128, 128], BF16, kind="ExternalInput")
    NEGMASK = nc.dram_tensor("NEGMASK", [128, KB * 128], BF16, kind="ExternalInput")
    LI = nc.dram_tensor("LI", [128, MT], F32, kind="ExternalOutput")

    with tile.TileContext(nc) as tc:
        with (
            tc.tile_pool(name="dram", bufs=1, space="DRAM") as dpool,
            tc.tile_pool(name="pre", bufs=7) as pre,
            tc.tile_pool(name="sqp", bufs=1) as sqp,
            tc.tile_pool(name="stat", bufs=8) as stat,
            tc.tile_pool(name="persist", bufs=1) as persist,
            tc.tile_pool(name="psum", bufs=3, space="PSUM") as psum_pool,
            tc.tile_pool(name="small", bufs=4) as small,
        ):
            XN8 = dpool.tile([N, D], FP8)
            QN8 = dpool.tile([QPC, D], FP8)

            ident = persist.tile([128, 128], BF16, tag="ident")
            nc.sync.dma_start(ident, IDENT[:, :])
            negmask = persist.tile([128, KB * 128], BF16, tag="negmask")
            nc.sync.dma_start(negmask, NEGMASK[:, :])

            # ---- normalize + quantize rows -> fp8 (scale 32/||x||) ----
            # reads batched 2 row-tiles per DMA (fewer SP-SEQ/HWDGE slots);
            # fp8 writes issue from the idle Pool sequencer (SWDGE) so their
            # waits never block the SP queue head.
            def norm_tiles(src, dst, ntiles, quant_dve=False,
                           square_dve=False):
                # fp8 outputs of two consecutive tiles share one SWDGE write
                # (halves Pool descriptor-generation work)
                assert ntiles % 2 == 0
                for t in range(ntiles):
                    xt = pre.tile([128, D], F32, tag="xt")
                    nc.sync.dma_start(xt, src[t * 128:(t + 1) * 128, :])
                    x8 = pre.tile([128, D], FP8, tag="x8")
                    sq = sqp.tile([128, D], F32, tag="sq")
                    n2 = stat.tile([128, 1], F32, tag="n2")
                    nc.scalar.activation(
                        sq, xt, mybir.ActivationFunctionType.Square,
                        accum_out=n2)
                    sn = stat.tile([128, 1], F32, tag="sn")
                    # sn = sqrt(n2/1024) = ||x||/32
                    nc.scalar.activation(
                        sn, n2, mybir.ActivationFunctionType.Sqrt,
                        scale=1.0 / (SCALE * SCALE))
                    rs = stat.tile([128, 1], F32, tag="rs")
                    nc.vector.reciprocal(rs, sn)
                    qeng = nc.vector if (quant_dve or t % 2 == 0) else nc.gpsimd
                    qeng.tensor_scalar(
                        x8, xt, rs, None, op0=mybir.AluOpType.mult)
                    nc.gpsimd.dma_start(dst[t * 128:(t + 1) * 128, :], x8)

            # query staging first (own slab), in a scoped pool so its SBUF is
            # released before the big accumulators allocate.
            QT8 = persist.tile([128, KC * 2 * QPC], FP8, tag="qt8")
            with tc.tile_pool(name="qstage", bufs=1) as qstage:
                norm_tiles(Q, QN8, QPC // 128, quant_dve=True, square_dve=True)
                qt_pk = qstage.tile([128, KC * QPC], U16, tag="qt_pk")
                qn8_u16 = QN8.bitcast(U16)
                for k in range(KC):
                    nc.sync.dma_start_transpose(
                        qt_pk[:, k * QPC:(k + 1) * QPC],
                        qn8_u16[:, k * 128:(k + 1) * 128],
                    )
                qtv = qt_pk.bitcast(FP8).rearrange(
                    "p (k m e) -> p k e m", k=KC, e=2)
                qt8v = QT8.rearrange("p (k e m) -> p k e m", k=KC, e=2)
                for k in range(KC):
                    for e in range(2):
                        nc.vector.tensor_copy(qt8v[:, k, e, :], qtv[:, k, e, :])

            # ---- normalize X and stage transposed keys, kb-granular ----
            XT = persist.tile([128, KC * N], U16, tag="xt_pk")
            xn8_u16 = XN8.bitcast(U16)

            def stage_kb(kb):
                norm_tiles(
                    X[kb * 2048:(kb + 1) * 2048, :],
                    XN8[kb * 2048:(kb + 1) * 2048, :],
                    2048 // 128,
                    quant_dve=(kb == 0),
                    square_dve=False,
                )
                for k in range(KC):
                    nc.sync.dma_start_transpose(
                        XT[:, k * N + kb * 2048: k * N + (kb + 1) * 2048],
                        xn8_u16[kb * 2048:(kb + 1) * 2048,
                                k * 128:(k + 1) * 128],
                    )

            # ---- main sweep ----
            ACC = persist.tile([128, MT * 1024], FP16, tag="acc")
            BM = persist.tile([128, MT * KB * 2], F32, tag="bm")
            nc.vector.memset(BM, -65000.0)
            SM = persist.tile([128, MT], F32, tag="sm")

            xtv = XT.bitcast(FP8).rearrange("p (k j e) -> p k e j", k=KC, e=2)
            qv = QT8.rearrange("p (k e m) -> p k e m", k=KC, e=2)

            first_act_done = [False] * MT
            last_act_kb = [max((kb for kb in range(KB) if not _is_direct(m, kb)),
                               default=-1) for m in range(MT)]

            with tc.tile_pool(name="cp", bufs=7) as cpool:
                stage_kb(0)
                for kb in range(KB):
                    # emit next staging ahead of this kb's sweep so its ops
                    # get earlier scheduler priority and overlap the sweep
                    # (one ahead: staging a kb is faster than sweeping one)
                    if kb + 1 < KB:
                        stage_kb(kb + 1)
                    for m in range(MT):
                        # two half-blocks of 1024 keys -> 4 psum bufs in
                        # flight (deeper MM/fold pipeline on the 8 banks)
                        for h in range(2):
                            ps = psum_pool.tile([128, 1024], F32, tag="ps")
                            for k in range(KC):
                                lhsT = qv[:, k, :, m * 128:(m + 1) * 128]
                                for nb in range(2):
                                    j0 = kb * 2048 + h * 1024 + nb * 512
                                    nc.tensor.matmul(
                                        ps[:, nb * 512:(nb + 1) * 512],
                                        lhsT,
                                        xtv[:, k, :, j0:j0 + 512],
                                        start=(k == 0),
                                        stop=(k == KC - 1),
                                        perf_mode=mybir.MatmulPerfMode.DoubleRow,
                                    )
                            # self-sim mask: adds NEGMASK[:, kb] (=-4096*I
                            # iff kb==c) at in-block cols [128*m, 128*m+128)
                            if (m * 128) // 1024 == h:
                                nc.tensor.matmul(
                                    ps[:, (m * 128) % 1024:(m * 128) % 1024 + 128],
                                    ident,
                                    negmask[:, kb * 128:(kb + 1) * 128],
                                    start=False,
                                    stop=True,
                                    skip_group_check=True,
                                )
                            if _is_direct(m, kb):
                                nc.vector.reduce_max(
                                    BM[:, (m * KB + kb) * 2 + h:
                                       (m * KB + kb) * 2 + h + 1], ps,
                                    axis=mybir.AxisListType.X)
                            elif not first_act_done[m]:
                                first_act_done[m] = True
                                nc.scalar.activation(
                                    ACC[:, m * 1024:(m + 1) * 1024], ps,
                                    mybir.ActivationFunctionType.Copy)
                            else:
                                cp = cpool.tile([128, 1024], FP16, tag="cp")
                                nc.scalar.activation(
                                    cp, ps, mybir.ActivationFunctionType.Copy)
                                nc.vector.tensor_tensor(
                                    out=ACC[:, m * 1024:(m + 1) * 1024],
                                    in0=cp,
                                    in1=ACC[:, m * 1024:(m + 1) * 1024],
                                    op=mybir.AluOpType.max)
                        if kb == last_act_kb[m]:
                            # acc complete for this m: reduce it now so the
                            # final reduces spread across the sweep
                            slot = (m * KB + kb) * 2
                            nc.vector.reduce_max(
                                BM[:, slot:slot + 1],
                                ACC[:, m * 1024:(m + 1) * 1024],
                                axis=mybir.AxisListType.X)
                        if kb == KB - 1:
                            # tiny: fold BM row (acc max lives in col 0 of
                            # this m's BM slice; direct cols fill the rest)
                            nc.vector.reduce_max(
                                SM[:, m:m + 1],
                                BM[:, m * KB * 2:(m + 1) * KB * 2],
                                axis=mybir.AxisListType.X)

            # dd = sqrt(2 - 2*smax/1024); li = log(dd + 1e-8)
            dd = persist.tile([128, MT], F32, tag="dd")
            b2 = persist.tile([128, 1], F32, tag="b2")
            nc.vector.memset(b2, 2.0)
            nc.scalar.activation(
                dd, SM, mybir.ActivationFunctionType.Sqrt,
                scale=-2.0 / DSCALE, bias=b2[:, 0:1])
            lg = persist.tile([128, MT], F32, tag="lg")
            beps = persist.tile([128, 1], F32, tag="beps")
            nc.vector.memset(beps, 1e-8)
            nc.scalar.activation(
                lg, dd, mybir.ActivationFunctionType.Ln, bias=beps[:, 0:1])
            nc.sync.dma_start(LI[:, :], lg)
    nc.compile()
    return nc


_CACHED = {}


def _get_nc():
    if "nc" not in _CACHED:
        _CACHED["nc"] = _build_nc()
    return _CACHED["nc"]


def kernel(X: np.ndarray) -> np.ndarray:
    global LAST_EXEC_NS
    X = np.ascontiguousarray(np.asarray(X, dtype=np.float32))
    assert X.shape == (N, D)

    nc = _get_nc()

    eye = np.eye(128, dtype=ml_dtypes.bfloat16)
    in_maps = []
    for c in range(NCORES):
        negmask = np.zeros((128, KB * 128), dtype=ml_dtypes.bfloat16)
        negmask[:, c * 128:(c + 1) * 128] = (
            np.eye(128) * -4096.0).astype(ml_dtypes.bfloat16)
        in_maps.append({
            "X": X,
            "Q": np.ascontiguousarray(X[c * QPC:(c + 1) * QPC]),
            "IDENT": eye,
            "NEGMASK": negmask,
        })

    res = run_bass_kernel_spmd(nc, in_maps, core_ids=list(range(NCORES)))
    LAST_EXEC_NS = res.exec_time_ns
    if LAST_EXEC_NS is None and "sim_ns" in _CACHED:
        LAST_EXEC_NS = _CACHED["sim_ns"]

    li = np.concatenate(
        [r["LI"].reshape(128, MT) for r in res.results], axis=1)
    loss = -np.float32(np.mean(li))
    return np.asarray(loss, dtype=np.float32)


def sim_exec_ns() -> float:
    """Single-core predicted duration from the TimelineSim cost model."""
    from concourse.timeline_sim import TimelineSim
    nc = _get_nc()
    sim = TimelineSim(nc, trace=False, no_exec=True)
    ns = sim.simulate()
    _CACHED["sim_ns"] = int(ns)
    return ns


if __name__ == "__main__":
    print("sim ns:", sim_exec_ns())


# revision 5
# speedup vs baseline: 1.0694x; 1.0098x over previous
"""KoLeo loss kernel for 8 trn2 NeuronCores — v2 (fp8 DoubleRow).

Math: L2-normalize rows of X [16384,768]; per row find max cosine sim
(self excluded); loss = -mean(log(sqrt(2-2*smax))).  Only the per-row max
dot is needed (indices never enter the loss).

Per core c (SPMD, core differences only via input data):
  - normalize all 16384 rows -> xn8 = fp8e4(32 * x / ||x||), staged to DRAM.
  - u16-bitcast DMA-transpose stages XT packed: partition p of 256-chunk k
    holds fp8 pair (d=256k+2p, 256k+2p+1) interleaved along free dim.
  - queries (rows [2048c, 2048(c+1)) passed as input Q) staged the same way,
    then deinterleaved on-chip to clean DoubleRow weight layout
    [128, k, e, m] (walrus rejects byte-paired weights; packed rhs is fine).
  - gram sweep: 16 m-tiles x 8 key-blocks of 2048; 3 DoubleRow fp8 matmuls
    x 4 n-blocks of 512 per block; dots scaled by 1024 in PSUM fp32.
  - self-sim removed by one extra bf16 matmul per (m,kb) adding NEGMASK
    (per-core input: -4096*I at kb==c else 0) at in-block offset 128*m.
  - row max: per (m,kb) either direct DVE reduce_max from PSUM, or ACT
    copy PSUM->SBUF fp16 then DVE tensor_tensor max into a per-m running
    acc (2x DVE mode).  Final per-m reduce + sqrt/log on device.
Host: loss = -mean(LI over all cores).
"""

import os

import ml_dtypes
import numpy as np

import concourse.bacc as bacc
import concourse.mybir as mybir
import concourse.tile as tile
from concourse.bass_utils import run_bass_kernel_spmd

F32 = mybir.dt.float32
BF16 = mybir.dt.bfloat16
FP16 = mybir.dt.float16
FP8 = mybir.dt.float8e4
U16 = mybir.dt.uint16

N = 16384
D = 768
NCORES = 8
QPC = N // NCORES          # 2048 queries per core
MT = QPC // 128            # 16 m-tiles
KB = N // 2048             # 8 key blocks of 2048
KC = 3                     # DoubleRow contraction chunks of 256
SCALE = 32.0               # fp8 quantization scale; dots scale by 1024
DSCALE = SCALE * SCALE

# per-(m,kb) fold path: direct-from-PSUM DVE reduce vs ACT copy + DVE
# tensor_tensor max into the per-m accumulator.  The direct fraction is
# skewed high for early kbs (DVE idles while ACT does staging squares) and
# low for late kbs (ACT frees up once staging ends, DVE owns the finals).
_NDIR = [8, 8, 7, 7, 7, 7, 6, 6]


def _is_direct(m, kb):
    return (m * 5 + kb) % 16 < _NDIR[kb]

LAST_EXEC_NS = None


def _build_nc():
    nc = bacc.Bacc("TRN2")

    X = nc.dram_tensor("X", [N, D], F32, kind="ExternalInput")
    Q = nc.dram_tensor("Q", [QPC, D], F32, kind="ExternalInput")
    IDENT = nc.dram_tensor("IDENT", [128, 128], BF16, kind="ExternalInput")
    NEGMASK = nc.dram_tensor("NEGMASK", [128, KB * 128], BF16, kind="ExternalInput")
    LI = nc.dram_tensor("LI", [128, MT], F32, kind="ExternalOutput")

    with tile.TileContext(nc) as tc:
        with (
            tc.tile_pool(name="dram", bufs=1, space="DRAM") as dpool,
            tc.tile_pool(name="pre", bufs=7) as pre,
            tc.tile_pool(name="sqp", bufs=1) as sqp,
            tc.tile_pool(name="stat", bufs=8) as stat,
            tc.tile_pool(name="persist", bufs=1) as persist,
            tc.tile_pool(name="psum", bufs=3, space="PSUM") as psum_pool,
            tc.tile_pool(name="small", bufs=4) as small,
        ):
            XN8 = dpool.tile([N, D], FP8)
            QN8 = dpool.tile([QPC, D], FP8)

            ident = persist.tile([128, 128], BF16, tag="ident")
            nc.sync.dma_start(ident, IDENT[:, :])
            negmask = persist.tile([128, KB * 128], BF16, tag="negmask")
            nc.sync.dma_start(negmask, NEGMASK[:, :])

            # ---- normalize + quantize rows -> fp8 (scale 32/||x||) ----
            # reads batched 2 row-tiles per DMA (fewer SP-SEQ/HWDGE slots);
            # fp8 writes issue from the idle Pool sequencer (SWDGE) so their
            # waits never block the SP queue head.
            def norm_tiles(src, dst, ntiles, quant_dve=False,
                           square_dve=False):
                # fp8 outputs of two consecutive tiles share one SWDGE write
                # (halves Pool descriptor-generation work)
                assert ntiles % 2 == 0
                x8pair = None
                for t in range(ntiles):
                    xt = pre.tile([128, D], F32, tag="xt")
                    nc.sync.dma_start(xt, src[t * 128:(t + 1) * 128, :])
                    if t % 4 == 0:
                        x8pair = pre.tile([128, 4, D], FP8, tag="x8")
                    x8 = x8pair[:, t % 4, :]
                    sq = sqp.tile([128, D], F32, tag="sq")
                    n2 = stat.tile([128, 1], F32, tag="n2")
                    nc.scalar.activation(
                        sq, xt, mybir.ActivationFunctionType.Square,
                        accum_out=n2)
                    sn = stat.tile([128, 1], F32, tag="sn")
                    # sn = sqrt(n2/1024) = ||x||/32
                    nc.scalar.activation(
                        sn, n2, mybir.ActivationFunctionType.Sqrt,
                        scale=1.0 / (SCALE * SCALE))
                    rs = stat.tile([128, 1], F32, tag="rs")
                    nc.vector.reciprocal(rs, sn)
                    qeng = nc.vector if (quant_dve or t % 2 == 0) else nc.gpsimd
                    qeng.tensor_scalar(
                        x8, xt, rs, None, op0=mybir.AluOpType.mult)
                    if t % 4 == 3:
                        nc.gpsimd.dma_start(
                            dst[(t - 3) * 128:(t + 1) * 128, :].rearrange(
                                "(r p) d -> p r d", p=128),
                            x8pair)

            # query staging first (own slab), in a scoped pool so its SBUF is
            # released before the big accumulators allocate.
            QT8 = persist.tile([128, KC * 2 * QPC], FP8, tag="qt8")
            with tc.tile_pool(name="qstage", bufs=1) as qstage:
                norm_tiles(Q, QN8, QPC // 128, quant_dve=True, square_dve=True)
                qt_pk = qstage.tile([128, KC * QPC], U16, tag="qt_pk")
                qn8_u16 = QN8.bitcast(U16)
                for k in range(KC):
                    nc.sync.dma_start_transpose(
                        qt_pk[:, k * QPC:(k + 1) * QPC],
                        qn8_u16[:, k * 128:(k + 1) * 128],
                    )
                qtv = qt_pk.bitcast(FP8).rearrange(
                    "p (k m e) -> p k e m", k=KC, e=2)
                qt8v = QT8.rearrange("p (k e m) -> p k e m", k=KC, e=2)
                for k in range(KC):
                    for e in range(2):
                        nc.vector.tensor_copy(qt8v[:, k, e, :], qtv[:, k, e, :])

            # ---- normalize X and stage transposed keys, kb-granular ----
            XT = persist.tile([128, KC * N], U16, tag="xt_pk")
            xn8_u16 = XN8.bitcast(U16)

            def stage_kb(kb):
                norm_tiles(
                    X[kb * 2048:(kb + 1) * 2048, :],
                    XN8[kb * 2048:(kb + 1) * 2048, :],
                    2048 // 128,
                    quant_dve=(kb == 0),
                    square_dve=False,
                )
                for k in range(KC):
                    nc.sync.dma_start_transpose(
                        XT[:, k * N + kb * 2048: k * N + (kb + 1) * 2048],
                        xn8_u16[kb * 2048:(kb + 1) * 2048,
                                k * 128:(k + 1) * 128],
                    )

            # ---- main sweep ----
            ACC = persist.tile([128, MT * 1024], FP16, tag="acc")
            BM = persist.tile([128, MT * KB * 2], F32, tag="bm")
            nc.vector.memset(BM, -65000.0)
            SM = persist.tile([128, MT], F32, tag="sm")

            xtv = XT.bitcast(FP8).rearrange("p (k j e) -> p k e j", k=KC, e=2)
            qv = QT8.rearrange("p (k e m) -> p k e m", k=KC, e=2)

            first_act_done = [False] * MT
            last_act_kb = [max((kb for kb in range(KB) if not _is_direct(m, kb)),
                               default=-1) for m in range(MT)]

            with tc.tile_pool(name="cp", bufs=7) as cpool:
                stage_kb(0)
                for kb in range(KB):
                    # emit next staging ahead of this kb's sweep so its ops
                    # get earlier scheduler priority and overlap the sweep
                    # (one ahead: staging a kb is faster than sweeping one)
                    if kb + 1 < KB:
                        stage_kb(kb + 1)
                    for m in range(MT):
                        # two half-blocks of 1024 keys -> 4 psum bufs in
                        # flight (deeper MM/fold pipeline on the 8 banks)
                        for h in range(2):
                            ps = psum_pool.tile([128, 1024], F32, tag="ps")
                            for k in range(KC):
                                lhsT = qv[:, k, :, m * 128:(m + 1) * 128]
                                for nb in range(2):
                                    j0 = kb * 2048 + h * 1024 + nb * 512
                                    nc.tensor.matmul(
                                        ps[:, nb * 512:(nb + 1) * 512],
                                        lhsT,
                                        xtv[:, k, :, j0:j0 + 512],
                                        start=(k == 0),
                                        stop=(k == KC - 1),
                                        perf_mode=mybir.MatmulPerfMode.DoubleRow,
                                    )
                            # self-sim mask: adds NEGMASK[:, kb] (=-4096*I
                            # iff kb==c) at in-block cols [128*m, 128*m+128)
                            if (m * 128) // 1024 == h:
                                nc.tensor.matmul(
                                    ps[:, (m * 128) % 1024:(m * 128) % 1024 + 128],
                                    ident,
                                    negmask[:, kb * 128:(kb + 1) * 128],
                                    start=False,
                                    stop=True,
                                    skip_group_check=True,
                                )
                            if _is_direct(m, kb):
                                nc.vector.reduce_max(
                                    BM[:, (m * KB + kb) * 2 + h:
                                       (m * KB + kb) * 2 + h + 1], ps,
                                    axis=mybir.AxisListType.X)
                            elif not first_act_done[m]:
                                first_act_done[m] = True
                                nc.scalar.activation(
                                    ACC[:, m * 1024:(m + 1) * 1024], ps,
                                    mybir.ActivationFunctionType.Copy)
                            else:
                                cp = cpool.tile([128, 1024], FP16, tag="cp")
                                nc.scalar.activation(
                                    cp, ps, mybir.ActivationFunctionType.Copy)
                                nc.vector.tensor_tensor(
                                    out=ACC[:, m * 1024:(m + 1) * 1024],
                                    in0=cp,
                                    in1=ACC[:, m * 1024:(m + 1) * 1024],
                                    op=mybir.AluOpType.max)
                        if kb == last_act_kb[m]:
                            # acc complete for this m: reduce it now so the
                            # final reduces spread across the sweep
                            slot = (m * KB + kb) * 2
                            nc.vector.reduce_max(
                                BM[:, slot:slot + 1],
                                ACC[:, m * 1024:(m + 1) * 1024],
                                axis=mybir.AxisListType.X)
                        if kb == KB - 1:
                            # tiny: fold BM row (acc max lives in col 0 of
                            # this m's BM slice; direct cols fill the rest)
                            nc.vector.reduce_max(
                                SM[:, m:m + 1],
                                BM[:, m * KB * 2:(m + 1) * KB * 2],
                                axis=mybir.AxisListType.X)

            # dd = sqrt(2 - 2*smax/1024); li = log(dd + 1e-8)
            dd = persist.tile([128, MT], F32, tag="dd")
            b2 = persist.tile([128, 1], F32, tag="b2")
            nc.vector.memset(b2, 2.0)
            nc.scalar.activation(
                dd, SM, mybir.ActivationFunctionType.Sqrt,
                scale=-2.0 / DSCALE, bias=b2[:, 0:1])
            lg = persist.tile([128, MT], F32, tag="lg")
            beps = persist.tile([128, 1], F32, tag="beps")
            nc.vector.memset(beps, 1e-8)
            nc.scalar.activation(
                lg, dd, mybir.ActivationFunctionType.Ln, bias=beps[:, 0:1])
            nc.sync.dma_start(LI[:, :], lg)
    nc.compile()
    return nc


_CACHED = {}


def _get_nc():
    if "nc" not in _CACHED:
        _CACHED["nc"] = _build_nc()
    return _CACHED["nc"]


def kernel(X: np.ndarray) -> np.ndarray:
    global LAST_EXEC_NS
    X = np.ascontiguousarray(np.asarray(X, dtype=np.float32))
    assert X.shape == (N, D)

    nc = _get_nc()

    eye = np.eye(128, dtype=ml_dtypes.bfloat16)
    in_maps = []
    for c in range(NCORES):
        negmask = np.zeros((128, KB * 128), dtype=ml_dtypes.bfloat16)
        negmask[:, c * 128:(c + 1) * 128] = (
            np.eye(128) * -4096.0).astype(ml_dtypes.bfloat16)
        in_maps.append({
            "X": X,
            "Q": np.ascontiguousarray(X[c * QPC:(c + 1) * QPC]),
            "IDENT": eye,
            "NEGMASK": negmask,
        })

    res = run_bass_kernel_spmd(nc, in_maps, core_ids=list(range(NCORES)))
    LAST_EXEC_NS = res.exec_time_ns
    if LAST_EXEC_NS is None and "sim_ns" in _CACHED:
        LAST_EXEC_NS = _CACHED["sim_ns"]

    li = np.concatenate(
        [r["LI"].reshape(128, MT) for r in res.results], axis=1)
    loss = -np.float32(np.mean(li))
    return np.asarray(loss, dtype=np.float32)


def sim_exec_ns() -> float:
    """Single-core predicted duration from the TimelineSim cost model."""
    from concourse.timeline_sim import TimelineSim
    nc = _get_nc()
    sim = TimelineSim(nc, trace=False, no_exec=True)
    ns = sim.simulate()
    _CACHED["sim_ns"] = int(ns)
    return ns


if __name__ == "__main__":
    print("sim ns:", sim_exec_ns())


# revision 6
# speedup vs baseline: 1.0788x; 1.0088x over previous
"""KoLeo loss kernel for 8 trn2 NeuronCores — v2 (fp8 DoubleRow).

Math: L2-normalize rows of X [16384,768]; per row find max cosine sim
(self excluded); loss = -mean(log(sqrt(2-2*smax))).  Only the per-row max
dot is needed (indices never enter the loss).

Per core c (SPMD, core differences only via input data):
  - normalize all 16384 rows -> xn8 = fp8e4(32 * x / ||x||), staged to DRAM.
  - u16-bitcast DMA-transpose stages XT packed: partition p of 256-chunk k
    holds fp8 pair (d=256k+2p, 256k+2p+1) interleaved along free dim.
  - queries (rows [2048c, 2048(c+1)) passed as input Q) staged the same way,
    then deinterleaved on-chip to clean DoubleRow weight layout
    [128, k, e, m] (walrus rejects byte-paired weights; packed rhs is fine).
  - gram sweep: 16 m-tiles x 8 key-blocks of 2048; 3 DoubleRow fp8 matmuls
    x 4 n-blocks of 512 per block; dots scaled by 1024 in PSUM fp32.
  - self-sim removed by one extra bf16 matmul per (m,kb) adding NEGMASK
    (per-core input: -4096*I at kb==c else 0) at in-block offset 128*m.
  - row max: per (m,kb) either direct DVE reduce_max from PSUM, or ACT
    copy PSUM->SBUF fp16 then DVE tensor_tensor max into a per-m running
    acc (2x DVE mode).  Final per-m reduce + sqrt/log on device.
Host: loss = -mean(LI over all cores).
"""

import os

import ml_dtypes
import numpy as np

import concourse.bacc as bacc
import concourse.mybir as mybir
import concourse.tile as tile
from concourse.bass_utils import run_bass_kernel_spmd

F32 = mybir.dt.float32
BF16 = mybir.dt.bfloat16
FP16 = mybir.dt.float16
FP8 = mybir.dt.float8e4
U16 = mybir.dt.uint16

N = 16384
D = 768
NCORES = 8
QPC = N // NCORES          # 2048 queries per core
MT = QPC // 128            # 16 m-tiles
KB = N // 2048             # 8 key blocks of 2048
KC = 3                     # DoubleRow contraction chunks of 256
SCALE = 32.0               # fp8 quantization scale; dots scale by 1024
DSCALE = SCALE * SCALE

# per-(m,kb) fold path: direct-from-PSUM DVE reduce vs ACT copy + DVE
# tensor_tensor max into the per-m accumulator.  The direct fraction is
# skewed high for early kbs (DVE idles while ACT does staging squares) and
# low for late kbs (ACT frees up once staging ends, DVE owns the finals).
_NDIR = [8, 8, 7, 7, 7, 7, 6, 6]


def _is_direct(m, kb):
    return (m * 5 + kb) % 16 < _NDIR[kb]

LAST_EXEC_NS = None


def _build_nc():
    nc = bacc.Bacc("TRN2")

    X = nc.dram_tensor("X", [N, D], F32, kind="ExternalInput")
    Q = nc.dram_tensor("Q", [QPC, D], F32, kind="ExternalInput")
    IDENT = nc.dram_tensor("IDENT", [128, 128], BF16, kind="ExternalInput")
    NEGMASK = nc.dram_tensor("NEGMASK", [128, KB * 128], BF16, kind="ExternalInput")
    LI = nc.dram_tensor("LI", [128, MT], F32, kind="ExternalOutput")

    with tile.TileContext(nc) as tc:
        with (
            tc.tile_pool(name="dram", bufs=1, space="DRAM") as dpool,
            tc.tile_pool(name="pre", bufs=5) as pre,
            tc.tile_pool(name="sqp", bufs=1) as sqp,
            tc.tile_pool(name="stat", bufs=8) as stat,
            tc.tile_pool(name="persist", bufs=1) as persist,
            tc.tile_pool(name="psum", bufs=3, space="PSUM") as psum_pool,
            tc.tile_pool(name="small", bufs=4) as small,
        ):
            XN8 = dpool.tile([N, D], FP8)
            QN8 = dpool.tile([QPC, D], FP8)

            ident = persist.tile([128, 128], BF16, tag="ident")
            nc.sync.dma_start(ident, IDENT[:, :])
            negmask = persist.tile([128, KB * 128], BF16, tag="negmask")
            nc.sync.dma_start(negmask, NEGMASK[:, :])

            # ---- normalize + quantize rows -> fp8 (scale 32/||x||) ----
            # reads batched 2 row-tiles per DMA (fewer SP-SEQ/HWDGE slots);
            # fp8 writes issue from the idle Pool sequencer (SWDGE) so their
            # waits never block the SP queue head.
            def norm_tiles(src, dst, ntiles, quant_dve=False,
                           square_dve=False):
                # fp8 outputs of two consecutive tiles share one SWDGE write
                # (halves Pool descriptor-generation work)
                assert ntiles % 2 == 0
                x8pair = None
                xtp = None
                for t in range(ntiles):
                    if t % 2 == 0:
                        xtp = pre.tile([128, 2, D], F32, tag="xt")
                        nc.sync.dma_start(
                            xtp,
                            src[t * 128:(t + 2) * 128, :].rearrange(
                                "(r p) d -> p r d", p=128))
                    xt = xtp[:, t % 2, :]
                    if t % 4 == 0:
                        x8pair = pre.tile([128, 4, D], FP8, tag="x8")
                    x8 = x8pair[:, t % 4, :]
                    sq = sqp.tile([128, D], F32, tag="sq")
                    n2 = stat.tile([128, 1], F32, tag="n2")
                    nc.scalar.activation(
                        sq, xt, mybir.ActivationFunctionType.Square,
                        accum_out=n2)
                    sn = stat.tile([128, 1], F32, tag="sn")
                    # sn = sqrt(n2/1024) = ||x||/32
                    nc.scalar.activation(
                        sn, n2, mybir.ActivationFunctionType.Sqrt,
                        scale=1.0 / (SCALE * SCALE))
                    rs = stat.tile([128, 1], F32, tag="rs")
                    nc.vector.reciprocal(rs, sn)
                    qeng = nc.vector if (quant_dve or t % 2 == 0) else nc.gpsimd
                    qeng.tensor_scalar(
                        x8, xt, rs, None, op0=mybir.AluOpType.mult)
                    if t % 4 == 3:
                        nc.gpsimd.dma_start(
                            dst[(t - 3) * 128:(t + 1) * 128, :].rearrange(
                                "(r p) d -> p r d", p=128),
                            x8pair)

            # query staging first (own slab), in a scoped pool so its SBUF is
            # released before the big accumulators allocate.
            QT8 = persist.tile([128, KC * 2 * QPC], FP8, tag="qt8")
            with tc.tile_pool(name="qstage", bufs=1) as qstage:
                norm_tiles(Q, QN8, QPC // 128, quant_dve=True, square_dve=True)
                qt_pk = qstage.tile([128, KC * QPC], U16, tag="qt_pk")
                qn8_u16 = QN8.bitcast(U16)
                for k in range(KC):
                    nc.sync.dma_start_transpose(
                        qt_pk[:, k * QPC:(k + 1) * QPC],
                        qn8_u16[:, k * 128:(k + 1) * 128],
                    )
                qtv = qt_pk.bitcast(FP8).rearrange(
                    "p (k m e) -> p k e m", k=KC, e=2)
                qt8v = QT8.rearrange("p (k e m) -> p k e m", k=KC, e=2)
                for k in range(KC):
                    for e in range(2):
                        nc.vector.tensor_copy(qt8v[:, k, e, :], qtv[:, k, e, :])

            # ---- normalize X and stage transposed keys, kb-granular ----
            XT = persist.tile([128, KC * N], U16, tag="xt_pk")
            xn8_u16 = XN8.bitcast(U16)

            def stage_kb(kb):
                norm_tiles(
                    X[kb * 2048:(kb + 1) * 2048, :],
                    XN8[kb * 2048:(kb + 1) * 2048, :],
                    2048 // 128,
                    quant_dve=(kb == 0),
                    square_dve=False,
                )
                for k in range(KC):
                    nc.sync.dma_start_transpose(
                        XT[:, k * N + kb * 2048: k * N + (kb + 1) * 2048],
                        xn8_u16[kb * 2048:(kb + 1) * 2048,
                                k * 128:(k + 1) * 128],
                    )

            # ---- main sweep ----
            ACC = persist.tile([128, MT * 1024], FP16, tag="acc")
            BM = persist.tile([128, MT * KB * 2], F32, tag="bm")
            nc.vector.memset(BM, -65000.0)
            SM = persist.tile([128, MT], F32, tag="sm")

            xtv = XT.bitcast(FP8).rearrange("p (k j e) -> p k e j", k=KC, e=2)
            qv = QT8.rearrange("p (k e m) -> p k e m", k=KC, e=2)

            first_act_done = [False] * MT
            last_act_kb = [max((kb for kb in range(KB) if not _is_direct(m, kb)),
                               default=-1) for m in range(MT)]

            with tc.tile_pool(name="cp", bufs=7) as cpool:
                stage_kb(0)
                for kb in range(KB):
                    # emit next staging ahead of this kb's sweep so its ops
                    # get earlier scheduler priority and overlap the sweep
                    # (one ahead: staging a kb is faster than sweeping one)
                    if kb + 1 < KB:
                        stage_kb(kb + 1)
                    for m in range(MT):
                        # two half-blocks of 1024 keys -> 4 psum bufs in
                        # flight (deeper MM/fold pipeline on the 8 banks)
                        for h in range(2):
                            ps = psum_pool.tile([128, 1024], F32, tag="ps")
                            for k in range(KC):
                                lhsT = qv[:, k, :, m * 128:(m + 1) * 128]
                                for nb in range(2):
                                    j0 = kb * 2048 + h * 1024 + nb * 512
                                    nc.tensor.matmul(
                                        ps[:, nb * 512:(nb + 1) * 512],
                                        lhsT,
                                        xtv[:, k, :, j0:j0 + 512],
                                        start=(k == 0),
                                        stop=(k == KC - 1),
                                        perf_mode=mybir.MatmulPerfMode.DoubleRow,
                                    )
                            # self-sim mask: adds NEGMASK[:, kb] (=-4096*I
                            # iff kb==c) at in-block cols [128*m, 128*m+128)
                            if (m * 128) // 1024 == h:
                                nc.tensor.matmul(
                                    ps[:, (m * 128) % 1024:(m * 128) % 1024 + 128],
                                    ident,
                                    negmask[:, kb * 128:(kb + 1) * 128],
                                    start=False,
                                    stop=True,
                                    skip_group_check=True,
                                )
                            if _is_direct(m, kb):
                                nc.vector.reduce_max(
                                    BM[:, (m * KB + kb) * 2 + h:
                                       (m * KB + kb) * 2 + h + 1], ps,
                                    axis=mybir.AxisListType.X)
                            elif not first_act_done[m]:
                                first_act_done[m] = True
                                nc.scalar.activation(
                                    ACC[:, m * 1024:(m + 1) * 1024], ps,
                                    mybir.ActivationFunctionType.Copy)
                            else:
                                cp = cpool.tile([128, 1024], FP16, tag="cp")
                                nc.scalar.activation(
                                    cp, ps, mybir.ActivationFunctionType.Copy)
                                nc.vector.tensor_tensor(
                                    out=ACC[:, m * 1024:(m + 1) * 1024],
                                    in0=cp,
                                    in1=ACC[:, m * 1024:(m + 1) * 1024],
                                    op=mybir.AluOpType.max)
                        if kb == last_act_kb[m]:
                            # acc complete for this m: reduce it now so the
                            # final reduces spread across the sweep
                            slot = (m * KB + kb) * 2
                            nc.vector.reduce_max(
                                BM[:, slot:slot + 1],
                                ACC[:, m * 1024:(m + 1) * 1024],
                                axis=mybir.AxisListType.X)
                        if kb == KB - 1:
                            # tiny: fold BM row (acc max lives in col 0 of
                            # this m's BM slice; direct cols fill the rest)
                            nc.vector.reduce_max(
                                SM[:, m:m + 1],
                                BM[:, m * KB * 2:(m + 1) * KB * 2],
                                axis=mybir.AxisListType.X)

            # dd = sqrt(2 - 2*smax/1024); li = log(dd + 1e-8)
            dd = persist.tile([128, MT], F32, tag="dd")
            b2 = persist.tile([128, 1], F32, tag="b2")
            nc.vector.memset(b2, 2.0)
            nc.scalar.activation(
                dd, SM, mybir.ActivationFunctionType.Sqrt,
                scale=-2.0 / DSCALE, bias=b2[:, 0:1])
            lg = persist.tile([128, MT], F32, tag="lg")
            beps = persist.tile([128, 1], F32, tag="beps")
            nc.vector.memset(beps, 1e-8)
            nc.scalar.activation(
                lg, dd, mybir.ActivationFunctionType.Ln, bias=beps[:, 0:1])
            nc.sync.dma_start(LI[:, :], lg)
    nc.compile()
    return nc


_CACHED = {}


def _get_nc():
    if "nc" not in _CACHED:
        _CACHED["nc"] = _build_nc()
    return _CACHED["nc"]


def kernel(X: np.ndarray) -> np.ndarray:
    global LAST_EXEC_NS
    X = np.ascontiguousarray(np.asarray(X, dtype=np.float32))
    assert X.shape == (N, D)

    nc = _get_nc()

    eye = np.eye(128, dtype=ml_dtypes.bfloat16)
    in_maps = []
    for c in range(NCORES):
        negmask = np.zeros((128, KB * 128), dtype=ml_dtypes.bfloat16)
        negmask[:, c * 128:(c + 1) * 128] = (
            np.eye(128) * -4096.0).astype(ml_dtypes.bfloat16)
        in_maps.append({
            "X": X,
            "Q": np.ascontiguousarray(X[c * QPC:(c + 1) * QPC]),
            "IDENT": eye,
            "NEGMASK": negmask,
        })

    res = run_bass_kernel_spmd(nc, in_maps, core_ids=list(range(NCORES)))
    LAST_EXEC_NS = res.exec_time_ns
    if LAST_EXEC_NS is None and "sim_ns" in _CACHED:
        LAST_EXEC_NS = _CACHED["sim_ns"]

    li = np.concatenate(
        [r["LI"].reshape(128, MT) for r in res.results], axis=1)
    loss = -np.float32(np.mean(li))
    return np.asarray(loss, dtype=np.float32)


def sim_exec_ns() -> float:
    """Single-core predicted duration from the TimelineSim cost model."""
    from concourse.timeline_sim import TimelineSim
    nc = _get_nc()
    sim = TimelineSim(nc, trace=False, no_exec=True)
    ns = sim.simulate()
    _CACHED["sim_ns"] = int(ns)
    return ns


if __name__ == "__main__":
    print("sim ns:", sim_exec_ns())
